# revision 3
# baseline (speedup 1.0000x reference)
"""Trainium2 Bass kernel for dual-input multi-head attention.

Computes, for each of two independent inputs x, y of shape [8, 1024, 768]:
    qkv = inp @ w_qkv.T ; split into 12 heads of 64
    attn = softmax(q k^T / sqrt(64)) v
    out  = attn @ w_proj.T + b_proj
Sharded data-parallel over the batch dim: core i handles batch i of x AND
batch i of y (16 batch-units over 8 cores = 2 per core).

Layout strategy (per core):
  - Host pre-transposes: inpT [C, N], w_qkvT [C, 3C], w_projT [C, C].
  - QKV matmuls produce q,k TRANSPOSED ([head_dim, N] per head, packed as
    12 j-tiles of [128, N]) and v in natural [N, head_dim] layout with a
    column of ones appended (so the P@V matmul also emits the softmax
    denominator as an extra output row for free).
  - Scores are computed transposed: pT[m, n] = k_m . q_n, so the two heads
    of a j-tile sit at partition halves 0-63 / 64-127 and their score
    matmuls (contraction dim = 64) row-pack onto the PE array concurrently.
  - exp via ScalarE reading PSUM directly (scale=1/sqrt(64) folded in, no
    max-subtraction: scores are O(+-15) so fp32 exp is exact enough).
  - out^T = [v|1]^T @ exp(pT) accumulated over key chunks -> rows 0..63 are
    the unnormalized attention output (already in the transposed layout the
    projection matmul needs), row 64 is the softmax denominator.
  - normalization: reciprocal of row 64, broadcast across partitions on the
    (otherwise idle) GpSimd engine, then one elementwise multiply.
  - projection + bias add, output in natural [N, C] layout.
All matmuls run as float32r (1 cycle/row at N>=256, ~TF32 accuracy); every
tile feeding a matmul is allocated as float32r so its producer rounds it
(walrus BIR verifier requirement).
"""

import numpy as np

import concourse.bacc as bacc
import concourse.mybir as mybir
import concourse.tile as tile
from concourse import bass_utils

B, N, C, H, HD = 8, 1024, 768, 12, 64
NT = N // 128  # 8 token tiles
CT = C // 128  # 6 contraction chunks
SCALE = HD ** -0.5
F32 = mybir.dt.float32
F32R = mybir.dt.float32r
AF = mybir.ActivationFunctionType
ALU = mybir.AluOpType
N_CORES = 8


def build_program():
    nc = bacc.Bacc("TRN2", target_bir_lowering=False, debug=False)
    # inputs carry float32 bits; declared f32r so DMA-filled SBUF tiles can
    # feed f32r matmuls directly (np binding treats f32r as float32)
    xT = nc.dram_tensor("xT", [C, N], F32R, kind="ExternalInput")
    yT = nc.dram_tensor("yT", [C, N], F32R, kind="ExternalInput")
    wqT = nc.dram_tensor("wqT", [C, 3 * C], F32R, kind="ExternalInput")
    wpT = nc.dram_tensor("wpT", [C, C], F32R, kind="ExternalInput")
    bp = nc.dram_tensor("bp", [1, C], F32R, kind="ExternalInput")
    out_x = nc.dram_tensor("out_x", [N, C], F32, kind="ExternalOutput")
    out_y = nc.dram_tensor("out_y", [N, C], F32, kind="ExternalOutput")

    with tile.TileContext(nc) as tc:
        with tc.tile_pool(name="pers", bufs=1) as pers:
            # memset can't write f32r directly: fill f32 scratch, copy-rounds
            onesf = pers.tile([128, 128], F32, name="onesf")
            nc.vector.memset(onesf[:], 1.0)
            ones_row = pers.tile([1, 128], F32R, name="ones_row")
            nc.vector.tensor_copy(ones_row[:], onesf[0:1, :])
            b_row = pers.tile([1, C], F32R, name="b_row")
            nc.sync.dma_start(b_row[:], bp[:, :])
            bias_sb = pers.tile([128, C], F32, name="bias_sb")
            # v values per (token-tile, head): 64 cols of v then one col of 1s
            v_sb = pers.tile([128, NT, H, HD + 1], F32R, name="v_sb")
            nc.vector.tensor_copy(v_sb[:, :, :, HD : HD + 1], onesf[:, 0 : NT * H])
            # q,k transposed: j-tiles 0..5 = q (2 heads per tile), 6..11 = k
            qkT_sb = pers.tile([128, H, N], F32R, name="qkT_sb")
            # attention output, transposed [C, N] as 6 chunks of 128
            attnT_sb = pers.tile([128, CT, N], F32R, name="attnT_sb")

            # broadcast bias across partitions via K=1 matmul
            with tc.tile_pool(name="bps", bufs=1, space="PSUM") as bps:
                for off, w in ((0, 512), (512, 256)):
                    bias_ps = bps.tile([128, 512], F32, name="bias_ps", tag="bps")
                    nc.tensor.matmul(
                        bias_ps[:, :w],
                        ones_row[:1, :],
                        b_row[:1, off : off + w],
                        start=True,
                        stop=True,
                    )
                    nc.vector.tensor_copy(bias_sb[:, off : off + w], bias_ps[:, :w])

            for idx, (inp, out_dram) in enumerate(((xT, out_x), (yT, out_y))):
                # ---- phase 1: QKV projection ----
                with (
                    tc.tile_pool(name=f"qkv{idx}", bufs=1) as qp,
                    tc.tile_pool(name=f"qkvps{idx}", bufs=4, space="PSUM") as qps,
                ):
                    wq_sb = qp.tile([128, CT, 3 * C], F32R, name="wq_sb")
                    for c in range(CT):
                        nc.sync.dma_start(wq_sb[:, c, :], wqT[c * 128 : (c + 1) * 128, :])
                    inp_sb = qp.tile([128, CT, N], F32R, name="inp_sb")
                    for c in range(CT):
                        nc.sync.dma_start(inp_sb[:, c, :], inp[c * 128 : (c + 1) * 128, :])
                    # q,k transposed: qkvT[j, n] = sum_c w_qkvT[c, j] inpT[c, n]
                    for jt in range(H):  # 12 j-tiles (q: 0..5, k: 6..11)
                        for g in range(2):
                            ps = qps.tile([128, 512], F32, name="ps_qk", tag="qkps")
                            for c in range(CT):
                                nc.tensor.matmul(
                                    ps[:],
                                    wq_sb[:, c, jt * 128 : (jt + 1) * 128],
                                    inp_sb[:, c, g * 512 : (g + 1) * 512],
                                    start=(c == 0),
                                    stop=(c == CT - 1),
                                )
                            nc.vector.tensor_copy(
                                qkT_sb[:, jt, g * 512 : (g + 1) * 512], ps[:]
                            )
                    # v natural: v[n, j] = sum_c inpT[c, n] w_qkvT[c, 2C + j]
                    for nt in range(NT):
                        for g, w in ((0, 512), (1, 256)):
                            ps = qps.tile([128, 512], F32, name="ps_v", tag="qkps")
                            for c in range(CT):
                                nc.tensor.matmul(
                                    ps[:, :w],
                                    inp_sb[:, c, nt * 128 : (nt + 1) * 128],
                                    wq_sb[:, c, 2 * C + g * 512 : 2 * C + g * 512 + w],
                                    start=(c == 0),
                                    stop=(c == CT - 1),
                                )
                            hview = ps[:, :w].rearrange("p (h d) -> p h d", d=HD)
                            nc.vector.tensor_copy(
                                v_sb[:, nt, g * 8 : g * 8 + w // HD, 0:HD], hview
                            )

                # ---- phase 2: attention per head-pair ----
                with (
                    tc.tile_pool(name=f"pexp{idx}", bufs=17) as pep,
                    tc.tile_pool(name=f"small{idx}", bufs=4) as smp,
                    tc.tile_pool(name=f"rbsb{idx}", bufs=4) as rbsbp,
                    tc.tile_pool(name=f"scps{idx}", bufs=3, space="PSUM") as scp,
                    tc.tile_pool(name=f"pvps{idx}", bufs=2, space="PSUM") as pvp,
                ):
                    for t in range(H // 2):
                        pexp_tiles = {}
                        for mt in range(NT):
                            for ab in range(2):
                                h = 2 * t + ab
                                pb = ab * 64
                                # scores^T[m, n] for key tile mt, both 512-col halves
                                sc = scp.tile([128, 2, 512], F32, name="sc", tag="sc")
                                for g in range(2):
                                    nc.tensor.matmul(
                                        sc[:, g, :],
                                        qkT_sb[pb : pb + 64, 6 + t, mt * 128 : (mt + 1) * 128],
                                        qkT_sb[pb : pb + 64, t, g * 512 : (g + 1) * 512],
                                        start=True,
                                        stop=True,
                                        tile_position=(pb, 0),
                                    )
                                pe = pep.tile([128, N], F32R, name="pe", tag="pexp")
                                nc.scalar.activation(
                                    pe[:],
                                    sc[:].rearrange("p a b -> p (a b)"),
                                    AF.Exp,
                                    scale=SCALE,
                                )
                                pexp_tiles[(h, mt)] = pe
                        for ab in range(2):
                            h = 2 * t + ab
                            hc, pb = h // 2, (h % 2) * 64
                            for g in range(2):
                                pv = pvp.tile([HD + 1, 512], F32, name="pv", tag="pv")
                                for mt in range(NT):
                                    nc.tensor.matmul(
                                        pv[:],
                                        v_sb[:, mt, h, :],
                                        pexp_tiles[(h, mt)][:, g * 512 : (g + 1) * 512],
                                        start=(mt == 0),
                                        stop=(mt == NT - 1),
                                    )
                                recip = smp.tile([1, 512], F32, name="recip", tag="recip")
                                nc.vector.reciprocal(recip[:], pv[HD : HD + 1, :])
                                rb_sb = rbsbp.tile([64, 512], F32, name="rb_sb", tag="rb")
                                nc.gpsimd.partition_broadcast(rb_sb[:], recip[:1, :])
                                nc.vector.tensor_tensor(
                                    attnT_sb[pb : pb + 64, hc, g * 512 : (g + 1) * 512],
                                    pv[0:HD, :],
                                    rb_sb[:],
                                    op=ALU.mult,
                                )

                # ---- phase 3: output projection ----
                with (
                    tc.tile_pool(name=f"proj{idx}", bufs=2) as pjp,
                    tc.tile_pool(name=f"prps{idx}", bufs=4, space="PSUM") as prp,
                ):
                    wp_sb = pjp.tile([128, CT, C], F32R, name="wp_sb", bufs=1)
                    for c in range(CT):
                        nc.sync.dma_start(wp_sb[:, c, :], wpT[c * 128 : (c + 1) * 128, :])
                    for nt in range(NT):
                        p1 = prp.tile([128, 512], F32, name="p1", tag="prps")
                        p2 = prp.tile([128, 512], F32, name="p2", tag="prps")
                        for c in range(CT):
                            nc.tensor.matmul(
                                p1[:],
                                attnT_sb[:, c, nt * 128 : (nt + 1) * 128],
                                wp_sb[:, c, 0:512],
                                start=(c == 0),
                                stop=(c == CT - 1),
                            )
                        for c in range(CT):
                            nc.tensor.matmul(
                                p2[:, :256],
                                attnT_sb[:, c, nt * 128 : (nt + 1) * 128],
                                wp_sb[:, c, 512:768],
                                start=(c == 0),
                                stop=(c == CT - 1),
                            )
                        out_sb = pjp.tile([128, C], F32, name="out_sb", tag="outsb")
                        nc.vector.tensor_tensor(out_sb[:, 0:512], p1[:], bias_sb[:, 0:512], op=ALU.add)
                        nc.vector.tensor_tensor(out_sb[:, 512:768], p2[:, :256], bias_sb[:, 512:768], op=ALU.add)
                        nc.sync.dma_start(out_dram[nt * 128 : (nt + 1) * 128, :], out_sb[:])

    nc.compile()
    return nc


_PROGRAM = None


def _get_program():
    global _PROGRAM
    if _PROGRAM is None:
        _PROGRAM = build_program()
    return _PROGRAM


def make_in_maps(x, y, w_qkv, w_proj, b_proj):
    x = np.asarray(x, np.float32)
    y = np.asarray(y, np.float32)
    xT = np.ascontiguousarray(x.transpose(0, 2, 1))
    yT = np.ascontiguousarray(y.transpose(0, 2, 1))
    wqT = np.ascontiguousarray(np.asarray(w_qkv, np.float32).T)
    wpT = np.ascontiguousarray(np.asarray(w_proj, np.float32).T)
    bp = np.ascontiguousarray(np.asarray(b_proj, np.float32).reshape(1, C))
    return [
        {"xT": xT[i], "yT": yT[i], "wqT": wqT, "wpT": wpT, "bp": bp}
        for i in range(N_CORES)
    ]


def kernel(x, y, w_qkv, w_proj, b_proj):
    nc = _get_program()
    in_maps = make_in_maps(x, y, w_qkv, w_proj, b_proj)
    res = bass_utils.run_bass_kernel_spmd(nc, in_maps, core_ids=list(range(N_CORES)))
    xo = np.stack([np.asarray(res.results[i]["out_x"]) for i in range(N_CORES)])
    yo = np.stack([np.asarray(res.results[i]["out_y"]) for i in range(N_CORES)])
    return (xo, yo)


# revision 9
# speedup vs baseline: 1.4265x; 1.4265x over previous
"""Trainium2 Bass kernel for dual-input multi-head attention.

Computes, for each of two independent inputs x, y of shape [8, 1024, 768]:
    qkv = inp @ w_qkv.T ; split into 12 heads of 64
    attn = softmax(q k^T / sqrt(64)) v
    out  = attn @ w_proj.T + b_proj
Sharded data-parallel over the batch dim: core i handles batch i of x AND
batch i of y (16 batch-units over 8 cores = 2 per core).

Per-core design:
  - Host pre-transposes and casts to bf16: inpT [C, N], w_qkvT [C, 3C],
    w_projT [C, C]. All matmuls run in bf16 (1 cycle/row on the PE) with
    fp32 PSUM accumulation.
  - QKV matmuls produce q,k TRANSPOSED ([head_dim, N] per head, as 12
    j-tiles of [128, N]) and v in natural [N, head_dim] layout with a
    column of ones appended, so the P@V matmul also emits the softmax
    denominator as a 65th output row for free.
  - Scores are computed transposed (pT[m, n] = k_m . q_n, contraction=64),
    exp on ScalarE straight out of PSUM (scale=1/sqrt(64) folded in; no
    max-subtraction — scores are O(+-15) so exp stays in fp32 range), and
    the P@V accumulation is interleaved with the score matmuls at key-tile
    granularity so the PE never idles long enough for the HAM clock gate
    to re-throttle it.
  - Normalization: denominators of a head-pair are batched into one [4,512]
    reciprocal (DVE reciprocal cost is per-free-element, so batching rows
    is ~4x cheaper), broadcast across partitions on the idle GpSimd engine,
    then one multiply into the transposed attention-output buffer -- which
    is exactly the lhsT layout the projection matmul wants.
  - Cross-input software pipelining: weights load once; QKV of input y is
    emitted between the attention head-pairs of input x (filling the PE
    bubbles left by the ScalarE-bound softmax), and the projection of x is
    emitted between the attention head-pairs of y.
"""

import numpy as np

import concourse.bacc as bacc
import concourse.mybir as mybir
import concourse.tile as tile
from concourse import bass_utils

B, N, C, H, HD = 8, 1024, 768, 12, 64
NT = N // 128  # 8 token tiles
CT = C // 128  # 6 contraction chunks
SCALE = HD ** -0.5
F32 = mybir.dt.float32
BF16 = mybir.dt.bfloat16
AF = mybir.ActivationFunctionType
ALU = mybir.AluOpType
N_CORES = 8


def build_program():
    nc = bacc.Bacc("TRN2", target_bir_lowering=False, debug=False)
    inp_dram = [
        nc.dram_tensor("xT", [C, N], BF16, kind="ExternalInput"),
        nc.dram_tensor("yT", [C, N], BF16, kind="ExternalInput"),
    ]
    wqT = nc.dram_tensor("wqT", [C, 3 * C], BF16, kind="ExternalInput")
    wpT = nc.dram_tensor("wpT", [C, C], BF16, kind="ExternalInput")
    bp = nc.dram_tensor("bp", [1, C], F32, kind="ExternalInput")
    out_dram = [
        nc.dram_tensor("out_x", [N, C], F32, kind="ExternalOutput"),
        nc.dram_tensor("out_y", [N, C], F32, kind="ExternalOutput"),
    ]

    with tile.TileContext(nc) as tc:
        with (
            tc.tile_pool(name="pers", bufs=1) as pers,
            tc.tile_pool(name="dbl", bufs=2) as dbl,
            tc.tile_pool(name="pexp", bufs=5) as pep,
            tc.tile_pool(name="pvu", bufs=5) as pvup,
            tc.tile_pool(name="small", bufs=2) as smp,
            tc.tile_pool(name="rbsb", bufs=2) as rbsbp,
            tc.tile_pool(name="outp", bufs=2) as outp,
            tc.tile_pool(name="scps", bufs=2, space="PSUM") as scp,
            tc.tile_pool(name="mmps", bufs=4, space="PSUM") as mmp,
        ):
            b_row = pers.tile([1, C], F32, name="b_row")
            nc.sync.dma_start(b_row[:], bp[:, :])
            bias_sb = pers.tile([128, C], F32, name="bias_sb")
            nc.gpsimd.partition_broadcast(bias_sb[:], b_row[:1, :])
            wq_sb = pers.tile([128, CT, 3 * C], BF16, name="wq_sb")
            for c in range(CT):
                nc.sync.dma_start(wq_sb[:, c, :], wqT[c * 128 : (c + 1) * 128, :])
            wp_sb = pers.tile([128, CT, C], BF16, name="wp_sb")
            for c in range(CT):
                nc.sync.dma_start(wp_sb[:, c, :], wpT[c * 128 : (c + 1) * 128, :])

            # double-buffered per-input tiles (x and y alive simultaneously)
            inp_sb, qkT_sb, v_sb, attnT_sb = {}, {}, {}, {}
            for idx in range(2):
                inp_sb[idx] = dbl.tile([128, CT, N], BF16, name="inp_sb", tag="inp")
                for c in range(CT):
                    nc.sync.dma_start(
                        inp_sb[idx][:, c, :], inp_dram[idx][c * 128 : (c + 1) * 128, :]
                    )
                # q,k transposed: j-tiles 0..5 = q (2 heads/tile), 6..11 = k
                qkT_sb[idx] = dbl.tile([128, H, N], BF16, name="qkT_sb", tag="qkT")
                # v per (token-tile, head): 64 cols of v then one col of ones
                v_sb[idx] = dbl.tile([128, NT, H, HD + 1], BF16, name="v_sb", tag="v")
                nc.vector.memset(v_sb[idx][:, :, :, HD : HD + 1], 1.0)
                # attention output, transposed [C, N] as 6 chunks of 128
                attnT_sb[idx] = dbl.tile([128, CT, N], BF16, name="attnT_sb", tag="attnT")

            def emit_qkT(idx, jts):
                # qkvT[j, n] = sum_c w_qkvT[c, j] inpT[c, n]
                for jt in jts:
                    for g in range(2):
                        ps = mmp.tile([128, 512], F32, name="ps_qk", tag="mm")
                        for c in range(CT):
                            nc.tensor.matmul(
                                ps[:],
                                wq_sb[:, c, jt * 128 : (jt + 1) * 128],
                                inp_sb[idx][:, c, g * 512 : (g + 1) * 512],
                                start=(c == 0),
                                stop=(c == CT - 1),
                            )
                        nc.vector.tensor_copy(
                            qkT_sb[idx][:, jt, g * 512 : (g + 1) * 512], ps[:]
                        )

            def emit_v(idx, chunks):
                # v[n, j] = sum_c inpT[c, n] w_qkvT[c, 2C + j]
                for nt, g in chunks:
                    w = 512 if g == 0 else 256
                    ps = mmp.tile([128, 512], F32, name="ps_v", tag="mm")
                    for c in range(CT):
                        nc.tensor.matmul(
                            ps[:, :w],
                            inp_sb[idx][:, c, nt * 128 : (nt + 1) * 128],
                            wq_sb[:, c, 2 * C + g * 512 : 2 * C + g * 512 + w],
                            start=(c == 0),
                            stop=(c == CT - 1),
                        )
                    hview = ps[:, :w].rearrange("p (h d) -> p h d", d=HD)
                    nc.vector.tensor_copy(
                        v_sb[idx][:, nt, g * 8 : g * 8 + w // HD, 0:HD], hview
                    )

            def emit_attn_pair(idx, t):
                pvu = {}
                for ab in range(2):
                    h = 2 * t + ab
                    pb = ab * 64
                    pv = [
                        mmp.tile([HD + 1, 512], F32, name="pv", tag="mm")
                        for _ in range(2)
                    ]
                    for mt in range(NT):
                        sc = scp.tile([128, 2, 512], F32, name="sc", tag="sc")
                        for g in range(2):
                            nc.tensor.matmul(
                                sc[:, g, :],
                                qkT_sb[idx][pb : pb + 64, 6 + t, mt * 128 : (mt + 1) * 128],
                                qkT_sb[idx][pb : pb + 64, t, g * 512 : (g + 1) * 512],
                                start=True,
                                stop=True,
                                tile_position=(pb, 0),
                            )
                        pe = pep.tile([128, N], BF16, name="pe", tag="pexp")
                        nc.scalar.activation(
                            pe[:],
                            sc[:].rearrange("p a b -> p (a b)"),
                            AF.Exp,
                            scale=SCALE,
                        )
                        # interleave P@V partial sums with the score stream
                        for g in range(2):
                            nc.tensor.matmul(
                                pv[g],
                                v_sb[idx][:, mt, h, :],
                                pe[:, g * 512 : (g + 1) * 512],
                                start=(mt == 0),
                                stop=(mt == NT - 1),
                            )
                    for g in range(2):
                        u = pvup.tile([HD + 1, 512], F32, name="pvu", tag="pvu")
                        nc.vector.tensor_copy(u[:], pv[g][:])
                        pvu[(ab, g)] = u
                for r, (ab, g) in enumerate(pvu):
                    h = 2 * t + ab
                    hc, pb = h // 2, (h % 2) * 64
                    recip = smp.tile([1, 512], F32, name="recip", tag="recip")
                    nc.vector.reciprocal(recip[:], pvu[(ab, g)][HD : HD + 1, :])
                    rb_sb = rbsbp.tile([64, 512], F32, name="rb_sb", tag="rb")
                    nc.gpsimd.partition_broadcast(rb_sb[:], recip[:1, :])
                    nc.vector.tensor_tensor(
                        attnT_sb[idx][pb : pb + 64, hc, g * 512 : (g + 1) * 512],
                        pvu[(ab, g)][0:HD, :],
                        rb_sb[:],
                        op=ALU.mult,
                    )

            def emit_proj(idx, nts):
                for nt in nts:
                    p1 = mmp.tile([128, 512], F32, name="p1", tag="mm")
                    p2 = mmp.tile([128, 512], F32, name="p2", tag="mm")
                    for c in range(CT):
                        nc.tensor.matmul(
                            p1[:],
                            attnT_sb[idx][:, c, nt * 128 : (nt + 1) * 128],
                            wp_sb[:, c, 0:512],
                            start=(c == 0),
                            stop=(c == CT - 1),
                        )
                    for c in range(CT):
                        nc.tensor.matmul(
                            p2[:, :256],
                            attnT_sb[idx][:, c, nt * 128 : (nt + 1) * 128],
                            wp_sb[:, c, 512:768],
                            start=(c == 0),
                            stop=(c == CT - 1),
                        )
                    out_sb = outp.tile([128, C], F32, name="out_sb", tag="outsb")
                    nc.vector.tensor_tensor(
                        out_sb[:, 0:512], p1[:], bias_sb[:, 0:512], op=ALU.add
                    )
                    nc.vector.tensor_tensor(
                        out_sb[:, 512:768], p2[:, :256], bias_sb[:, 512:768], op=ALU.add
                    )
                    nc.sync.dma_start(
                        out_dram[idx][nt * 128 : (nt + 1) * 128, :], out_sb[:]
                    )

            # ---- pipelined emission ----
            emit_qkT(0, range(H))
            v_chunks = [(nt, g) for g in range(2) for nt in range(NT)]
            emit_v(0, v_chunks)
            # attn(x) with QKV(y) woven into the ScalarE-bound gaps
            vy = [v_chunks[3 * t : 3 * t + 3] for t in range(6)]
            for t in range(H // 2):
                emit_attn_pair(0, t)
                emit_qkT(1, [t, 6 + t])
                emit_v(1, vy[t])
            # attn(y) with proj(x) woven in
            nts = [[0, 1], [2], [3], [4], [5], [6, 7]]
            for t in range(H // 2):
                emit_attn_pair(1, t)
                emit_proj(0, nts[t])
            emit_proj(1, range(NT))

    nc.compile()
    return nc


_PROGRAM = None


def _get_program():
    global _PROGRAM
    if _PROGRAM is None:
        _PROGRAM = build_program()
    return _PROGRAM


def make_in_maps(x, y, w_qkv, w_proj, b_proj):
    import ml_dtypes

    bf = ml_dtypes.bfloat16
    x = np.asarray(x, np.float32)
    y = np.asarray(y, np.float32)
    xT = np.ascontiguousarray(x.transpose(0, 2, 1)).astype(bf)
    yT = np.ascontiguousarray(y.transpose(0, 2, 1)).astype(bf)
    wqT = np.ascontiguousarray(np.asarray(w_qkv, np.float32).T).astype(bf)
    wpT = np.ascontiguousarray(np.asarray(w_proj, np.float32).T).astype(bf)
    bp = np.ascontiguousarray(np.asarray(b_proj, np.float32).reshape(1, C))
    return [
        {"xT": xT[i], "yT": yT[i], "wqT": wqT, "wpT": wpT, "bp": bp}
        for i in range(N_CORES)
    ]


def kernel(x, y, w_qkv, w_proj, b_proj):
    nc = _get_program()
    in_maps = make_in_maps(x, y, w_qkv, w_proj, b_proj)
    res = bass_utils.run_bass_kernel_spmd(nc, in_maps, core_ids=list(range(N_CORES)))
    xo = np.stack([np.asarray(res.results[i]["out_x"]) for i in range(N_CORES)])
    yo = np.stack([np.asarray(res.results[i]["out_y"]) for i in range(N_CORES)])
    return (xo, yo)


# revision 12
# speedup vs baseline: 1.5880x; 1.1132x over previous
"""Trainium2 Bass kernel for dual-input multi-head attention.

Computes, for each of two independent inputs x, y of shape [8, 1024, 768]:
    qkv = inp @ w_qkv.T ; split into 12 heads of 64
    attn = softmax(q k^T / sqrt(64)) v
    out  = attn @ w_proj.T + b_proj
Sharded data-parallel over the batch dim: core i handles batch i of x AND
batch i of y (16 batch-units over 8 cores = 2 per core).

Per-core design:
  - Host pre-transposes and casts to bf16: inpT [C, N], w_qkvT [C, 3C],
    w_projT [C, C]. All matmuls run in bf16 (1 cycle/row on the PE) with
    fp32 PSUM accumulation.
  - QKV matmuls produce q,k TRANSPOSED ([head_dim, N] per head, as 12
    j-tiles of [128, N]) and v in natural [N, head_dim] layout with a
    column of ones appended, so the P@V matmul also emits the softmax
    denominator as a 65th output row for free.
  - Scores are computed transposed (pT[m, n] = k_m . q_n, contraction=64),
    exp on ScalarE straight out of PSUM (scale=1/sqrt(64) folded in; no
    max-subtraction — scores are O(+-15) so exp stays in fp32 range), and
    the P@V accumulation is interleaved with the score matmuls at key-tile
    granularity so the PE never idles long enough for the HAM clock gate
    to re-throttle it.
  - Normalization: the 4 denominator rows of a head-pair are gathered at
    partitions 0/32/64/96 of one tile and inverted by a single DVE
    reciprocal (its cost is per-free-element, so batching rows is ~4x
    cheaper); each inverse is staged back to a partition-0 row (GpSimd
    partition_broadcast only reads partition 0 on hardware), broadcast on
    the idle GpSimd engine, and applied with one multiply into the
    transposed attention-output buffer — exactly the lhsT layout the
    projection matmul wants.
  - Cross-input software pipelining: weights load once; QKV work of input
    y is drained from a filler queue between the attention head-passes of
    input x (filling the PE bubbles left by the ScalarE-bound softmax),
    and the projection of x likewise fills the attention of y.
"""

from collections import deque

import numpy as np

import concourse.bacc as bacc
import concourse.mybir as mybir
import concourse.tile as tile
from concourse import bass_utils

B, N, C, H, HD = 8, 1024, 768, 12, 64
NT = N // 128  # 8 token tiles
CT = C // 128  # 6 contraction chunks
SCALE = HD ** -0.5
F32 = mybir.dt.float32
BF16 = mybir.dt.bfloat16
AF = mybir.ActivationFunctionType
ALU = mybir.AluOpType
N_CORES = 8


def build_program():
    nc = bacc.Bacc("TRN2", target_bir_lowering=False, debug=False)
    inp_dram = [
        nc.dram_tensor("xT", [C, N], BF16, kind="ExternalInput"),
        nc.dram_tensor("yT", [C, N], BF16, kind="ExternalInput"),
    ]
    wqT = nc.dram_tensor("wqT", [C, 3 * C], BF16, kind="ExternalInput")
    wpT = nc.dram_tensor("wpT", [C, C], BF16, kind="ExternalInput")
    bp = nc.dram_tensor("bp", [1, C], F32, kind="ExternalInput")
    out_dram = [
        nc.dram_tensor("out_x", [N, C], F32, kind="ExternalOutput"),
        nc.dram_tensor("out_y", [N, C], F32, kind="ExternalOutput"),
    ]

    with tile.TileContext(nc) as tc:
        with (
            tc.tile_pool(name="pers", bufs=1) as pers,
            tc.tile_pool(name="dbl", bufs=2) as dbl,
            tc.tile_pool(name="pexp", bufs=5) as pep,
            tc.tile_pool(name="pvu", bufs=5) as pvup,
            tc.tile_pool(name="small", bufs=1) as smp,
            tc.tile_pool(name="rbsb", bufs=2) as rbsbp,
            tc.tile_pool(name="outp", bufs=2) as outp,
            tc.tile_pool(name="scps", bufs=2, space="PSUM") as scp,
            tc.tile_pool(name="mmps", bufs=4, space="PSUM") as mmp,
        ):
            # startup-critical DMAs first: interleave wq and x chunks so the
            # first QKV matmul group (needs wq[0] + x[0]) starts ASAP
            wq_sb = pers.tile([128, CT, 3 * C], BF16, name="wq_sb")
            inp_sb = {
                0: dbl.tile([128, CT, N], BF16, name="inp_sb", tag="inp"),
                1: dbl.tile([128, CT, N], BF16, name="inp_sb2", tag="inp"),
            }
            for c in range(CT):
                nc.sync.dma_start(wq_sb[:, c, :], wqT[c * 128 : (c + 1) * 128, :])
                nc.sync.dma_start(
                    inp_sb[0][:, c, :], inp_dram[0][c * 128 : (c + 1) * 128, :]
                )
            for c in range(CT):
                nc.sync.dma_start(
                    inp_sb[1][:, c, :], inp_dram[1][c * 128 : (c + 1) * 128, :]
                )
            wp_sb = pers.tile([128, CT, C], BF16, name="wp_sb")
            for c in range(CT):
                nc.sync.dma_start(wp_sb[:, c, :], wpT[c * 128 : (c + 1) * 128, :])
            b_row = pers.tile([1, C], F32, name="b_row")
            nc.sync.dma_start(b_row[:], bp[:, :])
            bias_sb = pers.tile([128, C], F32, name="bias_sb")
            nc.gpsimd.partition_broadcast(bias_sb[:], b_row[:1, :])

            qkT_sb, v_sb, attnT_sb = {}, {}, {}
            for idx in range(2):
                # q,k transposed: j-tiles 0..5 = q (2 heads/tile), 6..11 = k
                qkT_sb[idx] = dbl.tile([128, H, N], BF16, name="qkT_sb", tag="qkT")
                # v per (token-tile, head): 64 cols of v then one col of ones
                v_sb[idx] = dbl.tile([128, NT, H, HD + 1], BF16, name="v_sb", tag="v")
                nc.vector.memset(v_sb[idx][:, :, :, HD : HD + 1], 1.0)
                # attention output, transposed [C, N] as 6 chunks of 128
                attnT_sb[idx] = dbl.tile([128, CT, N], BF16, name="attnT_sb", tag="attnT")

            def emit_qkT(idx, jt, copy_engine):
                # qkvT[j, n] = sum_c w_qkvT[c, j] inpT[c, n]
                for g in range(2):
                    ps = mmp.tile([128, 512], F32, name="ps_qk", tag="mm")
                    for c in range(CT):
                        nc.tensor.matmul(
                            ps[:],
                            wq_sb[:, c, jt * 128 : (jt + 1) * 128],
                            inp_sb[idx][:, c, g * 512 : (g + 1) * 512],
                            start=(c == 0),
                            stop=(c == CT - 1),
                        )
                    dst = qkT_sb[idx][:, jt, g * 512 : (g + 1) * 512]
                    if copy_engine == "act":
                        nc.scalar.copy(dst, ps[:])
                    else:
                        nc.vector.tensor_copy(dst, ps[:])

            def emit_v(idx, nt, g, copy_engine):
                # v[n, j] = sum_c inpT[c, n] w_qkvT[c, 2C + j]
                w = 512 if g == 0 else 256
                ps = mmp.tile([128, 512], F32, name="ps_v", tag="mm")
                for c in range(CT):
                    nc.tensor.matmul(
                        ps[:, :w],
                        inp_sb[idx][:, c, nt * 128 : (nt + 1) * 128],
                        wq_sb[:, c, 2 * C + g * 512 : 2 * C + g * 512 + w],
                        start=(c == 0),
                        stop=(c == CT - 1),
                    )
                hview = ps[:, :w].rearrange("p (h d) -> p h d", d=HD)
                dst = v_sb[idx][:, nt, g * 8 : g * 8 + w // HD, 0:HD]
                if copy_engine == "act":
                    nc.scalar.copy(dst, hview)
                else:
                    nc.vector.tensor_copy(dst, hview)

            def emit_proj(idx, nt):
                p1 = mmp.tile([128, 512], F32, name="p1", tag="mm")
                p2 = mmp.tile([128, 512], F32, name="p2", tag="mm")
                for c in range(CT):
                    nc.tensor.matmul(
                        p1[:],
                        attnT_sb[idx][:, c, nt * 128 : (nt + 1) * 128],
                        wp_sb[:, c, 0:512],
                        start=(c == 0),
                        stop=(c == CT - 1),
                    )
                for c in range(CT):
                    nc.tensor.matmul(
                        p2[:, :256],
                        attnT_sb[idx][:, c, nt * 128 : (nt + 1) * 128],
                        wp_sb[:, c, 512:768],
                        start=(c == 0),
                        stop=(c == CT - 1),
                    )
                out_sb = outp.tile([128, C], F32, name="out_sb", tag="outsb")
                nc.vector.tensor_tensor(
                    out_sb[:, 0:512], p1[:], bias_sb[:, 0:512], op=ALU.add
                )
                nc.vector.tensor_tensor(
                    out_sb[:, 512:768], p2[:, :256], bias_sb[:, 512:768], op=ALU.add
                )
                nc.sync.dma_start(out_dram[idx][nt * 128 : (nt + 1) * 128, :], out_sb[:])

            fillers = deque()

            def drain(k):
                for _ in range(min(k, len(fillers))):
                    fillers.popleft()()

            def emit_attn_pair(idx, t):
                pvu = {}
                for ab in range(2):
                    h = 2 * t + ab
                    pb = ab * 64
                    pv = [
                        mmp.tile([HD + 1, 512], F32, name="pv", tag="mm")
                        for _ in range(2)
                    ]
                    for mt in range(NT):
                        sc = scp.tile([128, 2, 512], F32, name="sc", tag="sc")
                        for g in range(2):
                            nc.tensor.matmul(
                                sc[:, g, :],
                                qkT_sb[idx][pb : pb + 64, 6 + t, mt * 128 : (mt + 1) * 128],
                                qkT_sb[idx][pb : pb + 64, t, g * 512 : (g + 1) * 512],
                                start=True,
                                stop=True,
                                tile_position=(pb, 0),
                            )
                        pe = pep.tile([128, N], BF16, name="pe", tag="pexp")
                        nc.scalar.activation(
                            pe[:],
                            sc[:].rearrange("p a b -> p (a b)"),
                            AF.Exp,
                            scale=SCALE,
                        )
                        # interleave P@V partial sums with the score stream
                        for g in range(2):
                            nc.tensor.matmul(
                                pv[g],
                                v_sb[idx][:, mt, h, :],
                                pe[:, g * 512 : (g + 1) * 512],
                                start=(mt == 0),
                                stop=(mt == NT - 1),
                            )
                    for g in range(2):
                        u = pvup.tile([HD + 1, 512], F32, name="pvu", tag="pvu")
                        nc.vector.tensor_copy(u[:], pv[g][:])
                        pvu[(ab, g)] = u
                    drain(1)  # PE filler while ScalarE works on the next pass
                # batched softmax denominators: gather at partitions 0/32/64/96,
                # one reciprocal, stage each row back to partition 0 (GpSimd
                # partition_broadcast only reads partition 0 on HW)
                keys = list(pvu)
                sums4 = smp.tile([128, 512], F32, name="sums4", tag="sums")
                nc.vector.memset(sums4[:], 1.0)
                for r, k in enumerate(keys):
                    nc.vector.tensor_copy(
                        sums4[32 * r : 32 * r + 1, :], pvu[k][HD : HD + 1, :]
                    )
                recip4 = smp.tile([128, 512], F32, name="recip4", tag="recip")
                nc.vector.reciprocal(recip4[0:97, :], sums4[0:97, :])
                for r, (ab, g) in enumerate(keys):
                    h = 2 * t + ab
                    hc, pb = h // 2, (h % 2) * 64
                    if r == 0:
                        stage = recip4
                    else:
                        stage = smp.tile([1, 512], F32, name=f"st{r}", tag=f"st{r}")
                        nc.vector.tensor_copy(stage[0:1, :], recip4[32 * r : 32 * r + 1, :])
                    rb_sb = rbsbp.tile([64, 512], F32, name="rb_sb", tag="rb")
                    nc.gpsimd.partition_broadcast(rb_sb[:], stage[0:1, :])
                    nc.vector.tensor_tensor(
                        attnT_sb[idx][pb : pb + 64, hc, g * 512 : (g + 1) * 512],
                        pvu[(ab, g)][0:HD, :],
                        rb_sb[:],
                        op=ALU.mult,
                    )
                drain(1)

            # ---- pipelined emission ----
            for jt in range(H):
                emit_qkT(0, jt, "act")
            for g in range(2):
                for nt in range(NT):
                    emit_v(0, nt, g, "act")
            # attn(x) with QKV(y) drained into the ScalarE-bound gaps
            for jt in range(H):
                fillers.append(lambda jt=jt: emit_qkT(1, jt, "dve"))
            for g in range(2):
                for nt in range(NT):
                    fillers.append(lambda nt=nt, g=g: emit_v(1, nt, g, "dve"))
            for t in range(H // 2):
                emit_attn_pair(0, t)
            drain(len(fillers))
            # attn(y) with proj(x) drained in
            for nt in range(NT):
                fillers.append(lambda nt=nt: emit_proj(0, nt))
            for t in range(H // 2):
                emit_attn_pair(1, t)
            drain(len(fillers))
            for nt in range(NT):
                emit_proj(1, nt)

    nc.compile()
    return nc


_PROGRAM = None


def _get_program():
    global _PROGRAM
    if _PROGRAM is None:
        _PROGRAM = build_program()
    return _PROGRAM


def make_in_maps(x, y, w_qkv, w_proj, b_proj):
    import ml_dtypes

    bf = ml_dtypes.bfloat16
    x = np.asarray(x, np.float32)
    y = np.asarray(y, np.float32)
    xT = np.ascontiguousarray(x.transpose(0, 2, 1)).astype(bf)
    yT = np.ascontiguousarray(y.transpose(0, 2, 1)).astype(bf)
    wqT = np.ascontiguousarray(np.asarray(w_qkv, np.float32).T).astype(bf)
    wpT = np.ascontiguousarray(np.asarray(w_proj, np.float32).T).astype(bf)
    bp = np.ascontiguousarray(np.asarray(b_proj, np.float32).reshape(1, C))
    return [
        {"xT": xT[i], "yT": yT[i], "wqT": wqT, "wpT": wpT, "bp": bp}
        for i in range(N_CORES)
    ]


def kernel(x, y, w_qkv, w_proj, b_proj):
    nc = _get_program()
    in_maps = make_in_maps(x, y, w_qkv, w_proj, b_proj)
    res = bass_utils.run_bass_kernel_spmd(nc, in_maps, core_ids=list(range(N_CORES)))
    xo = np.stack([np.asarray(res.results[i]["out_x"]) for i in range(N_CORES)])
    yo = np.stack([np.asarray(res.results[i]["out_y"]) for i in range(N_CORES)])
    return (xo, yo)
